# revision 9
# baseline (speedup 1.0000x reference)
"""BoxFilter (9x9 mean with clamped borders) for Trainium2, 8 NeuronCores.

Input x: (16, 3, 1024, 1024) fp32. Output: same shape.
out[b,c,i,j] = mean of x[b,c, i-4:i+5, j-4:j+5] clipped to the image.

Strategy (per core, data-parallel over the 48 (b,c) images -> 6 images/core):
  - Load 128-row blocks as fp16 (cast during SWDGE DMA), zero-padded cols.
  - W-axis 9-window sum via shift-add tree on VectorE (fp16 keeps the 2x
    perf mode; all shifts even): e2 = {0,2,4,6} taps, e3 = {0,2,4,6,8} taps.
    window9[j] = e3[j] + e2[j+1]  -- the odd shift is folded into the matmul
    moving-operand slice.
  - H-axis 9-window sum as banded-matrix matmuls on TensorE: out block I
    accumulates W_prev @ eX[I-1] + W_mid @ eX[I] + W_next @ eX[I+1] in PSUM
    (weights are exact 0/1 bands in fp16).
  - PSUM -> SBUF copy on ScalarE applies the per-row 1/(ch*9) normalization
    (per-partition fp32 scale vector) and casts to fp16 for the store.
  - Host: upcast to fp32 and rescale the 4 left/right edge columns by 9/cw.
"""

import threading

import numpy as np

NCORES = 8
B, C, H, W = 16, 3, 1024, 1024
IMGS = B * C  # 48
IMGS_PER_CORE = IMGS // NCORES  # 6
NBLK = H // 128  # 8
R = 4  # window radius


def _window_counts():
    r = np.arange(H)
    return (np.minimum(r + R, H - 1) - np.maximum(r - R, 0) + 1).astype(np.float32)


def _consts():
    ch = _window_counts()  # 5..9

    k = np.arange(128)[:, None]
    m = np.arange(128)[None, :]
    w_mid = (np.abs(m - k) <= R).astype(np.float32)
    w_prev = ((k - m) >= 128 - R).astype(np.float32)
    w_next = ((m - k) >= 128 - R).astype(np.float32)
    wts = np.stack([w_prev, w_mid, w_next]).astype(np.float16)

    rowscale = np.empty((128, 3), np.float32)
    rowscale[:, 0] = 1.0 / (ch[0:128] * 9.0)
    rowscale[:, 1] = 1.0 / (ch[H - 128 : H] * 9.0)
    rowscale[:, 2] = 1.0 / 81.0
    return wts, rowscale


def _build(reps: int = 1):
    import concourse.bacc as bacc
    import concourse.mybir as mybir
    import concourse.tile as tile

    f32 = mybir.dt.float32
    f16 = mybir.dt.float16

    nc = bacc.Bacc("TRN2", target_bir_lowering=False, debug=False, num_devices=NCORES)
    x_d = nc.declare_dram_parameter("x", [IMGS_PER_CORE, H, W], f32, isOutput=False)
    wts_d = nc.declare_dram_parameter("wts", [3, 128, 128], f16, isOutput=False)
    rs_d = nc.declare_dram_parameter("rowscale", [128, 3], f32, isOutput=False)
    o_d = nc.declare_dram_parameter("out", [IMGS_PER_CORE, H, W], f16, isOutput=True)

    WPAD = W + 16  # 4 zero cols left, data, 12 zero cols right

    with tile.TileContext(nc) as tc:
        with (
            tc.tile_pool(name="consts", bufs=1) as cpool,
            tc.tile_pool(name="xb", bufs=4) as xb_pool,
            tc.tile_pool(name="e1", bufs=4) as e1_pool,
            tc.tile_pool(name="e2", bufs=8) as e2_pool,
            tc.tile_pool(name="e3", bufs=8) as e3_pool,
            tc.tile_pool(name="osb", bufs=4) as out_pool,
            tc.tile_pool(name="ps", bufs=8, space="PSUM") as ps_pool,
        ):
            w_sb = cpool.tile([128, 3 * 128], f16)
            for i in range(3):
                nc.sync.dma_start(out=w_sb[:, 128 * i : 128 * (i + 1)], in_=wts_d[i])
            rs_sb = cpool.tile([128, 3], f32)
            nc.sync.dma_start(out=rs_sb[:], in_=rs_d[:])

            def tree(g, J):
                xb = xb_pool.tile([128, WPAD], f16, tag="xb")
                nc.gpsimd.memset(xb[:, 0:4], 0.0)
                nc.gpsimd.memset(xb[:, W + 4 : WPAD], 0.0)
                nc.gpsimd.dma_start(
                    out=xb[:, 4 : W + 4], in_=x_d[g, 128 * J : 128 * (J + 1), :]
                )
                e1 = e1_pool.tile([128, WPAD - 2], f16, tag="e1")
                nc.vector.tensor_add(
                    out=e1[:], in0=xb[:, 0 : WPAD - 2], in1=xb[:, 2:WPAD]
                )
                e2 = e2_pool.tile([128, WPAD - 6], f16, tag="e2")
                nc.vector.tensor_add(
                    out=e2[:], in0=e1[:, 0 : WPAD - 6], in1=e1[:, 4 : WPAD - 2]
                )
                e3 = e3_pool.tile([128, WPAD - 14], f16, tag="e3")
                nc.vector.tensor_add(
                    out=e3[:], in0=e2[:, 0 : WPAD - 14], in1=xb[:, 8 : WPAD - 6]
                )
                return e2, e3

            def emit(g, I, streams):
                out_sb = out_pool.tile([128, W], f16, tag="osb")
                Js = [J for J in (I - 1, I, I + 1) if 0 <= J < NBLK]
                for h in range(2):
                    j0 = 512 * h
                    ps = ps_pool.tile([128, 512], f32, tag="ps")
                    n_mm = 2 * len(Js)
                    mm = 0
                    for J in Js:
                        e2, e3 = streams[J]
                        wv = w_sb[:, 128 * (J - I + 1) : 128 * (J - I + 2)]
                        nc.tensor.matmul(
                            ps[:],
                            wv,
                            e3[:, j0 : j0 + 512],
                            start=(mm == 0),
                            stop=(mm == n_mm - 1),
                        )
                        mm += 1
                        nc.tensor.matmul(
                            ps[:],
                            wv,
                            e2[:, j0 + 1 : j0 + 513],
                            start=False,
                            stop=(mm == n_mm - 1),
                        )
                        mm += 1
                    si = 0 if I == 0 else (1 if I == NBLK - 1 else 2)
                    nc.scalar.mul(
                        out_sb[:, j0 : j0 + 512], ps[:], rs_sb[:, si : si + 1]
                    )
                nc.scalar.dma_start(
                    out=o_d[g, 128 * I : 128 * (I + 1), :], in_=out_sb[:]
                )

            for _ in range(reps):
                for g in range(IMGS_PER_CORE):
                    streams = {}
                    for J in range(NBLK):
                        streams[J] = tree(g, J)
                        if J >= 1:
                            emit(g, J - 1, streams)
                    emit(g, NBLK - 1, streams)

    nc.compile()
    return nc


_LOCK = threading.Lock()
_CACHED = {}


def _get_nc(reps: int = 1):
    with _LOCK:
        key = ("nc", reps)
        if key not in _CACHED:
            _CACHED[key] = _build(reps)
        return _CACHED[key]


def _postprocess(out48_f16: np.ndarray) -> np.ndarray:
    """fp16 device output -> fp32 full output with edge-column rescale."""
    out = out48_f16.astype(np.float32).reshape(B, C, H, W)
    ch = _window_counts()
    out[..., 0:R] *= (9.0 / ch[0:R])[None, None, None, :]
    out[..., W - R : W] *= (9.0 / ch[H - R : H])[None, None, None, :]
    return out


def run(x: np.ndarray, trace: bool = False, reps: int = 1):
    """Run the 8-core kernel on full input x; returns (out, BassKernelResults)."""
    from concourse.bass_utils import run_bass_kernel_spmd

    assert x.shape == (B, C, H, W), x.shape
    x48 = np.ascontiguousarray(x.reshape(IMGS, H, W), dtype=np.float32)
    wts, rowscale = _consts()
    in_maps = [
        {
            "x": np.ascontiguousarray(
                x48[IMGS_PER_CORE * c : IMGS_PER_CORE * (c + 1)]
            ),
            "wts": wts,
            "rowscale": rowscale,
        }
        for c in range(NCORES)
    ]
    nc = _get_nc(reps)
    res = run_bass_kernel_spmd(
        nc, in_maps, core_ids=list(range(NCORES)), trace=trace
    )
    out48 = np.concatenate([r["out"] for r in res.results], axis=0)
    return _postprocess(out48), res


def kernel(x: np.ndarray) -> np.ndarray:
    out, _ = run(x, trace=False)
    return out


# revision 10
# speedup vs baseline: 1.2618x; 1.2618x over previous
"""BoxFilter 9x9 mean, TRN2 x8 — v5: overlapping input tiles, no neighbor MMs.

Each output block of <=120 rows is produced from ONE 128-row (or smaller,
clamped at image edges) input tile that already contains the +/-4-row halo.
Per psum half just 3 matmuls against one band weight: rhs = e2[j], e2[j+1]
(odd shift folded into the slice), xb[j+8]. DVE tree is only 2 ops
(e1={0,2}, e2={0,2,4,6}). One [rows,1024] 2-bank PSUM tile per block, one
ScalarE normalize+cast copy, one output DMA.
"""

import threading

import numpy as np

NCORES = 8
B, C, H, W = 16, 3, 1024, 1024
IMGS = B * C
IMGS_PER_CORE = IMGS // NCORES
R = 4
OB = 120  # output rows per full block
NFULL = H // OB  # 8 full blocks
LASTO = H - NFULL * OB  # 64
WPAD = W + 16

# per-image block table: (out_start, out_rows, in_start, in_rows, w_idx, rs_idx)
BLOCKS = []
BLOCKS.append((0, OB, 0, 128, 0, 0))
for I in range(1, NFULL):
    BLOCKS.append((OB * I, OB, OB * I - R, 128, 1, 2))
BLOCKS.append((H - LASTO, LASTO, H - 128, 128, 2, 1))


def _window_counts():
    r = np.arange(H)
    return (np.minimum(r + R, H - 1) - np.maximum(r - R, 0) + 1).astype(np.float32)


def _consts():
    ch = _window_counts()
    k = np.arange(128)[:, None]
    m = np.arange(128)[None, :]
    # W0: tile rows = image rows 0..127; out m needs rows max(0,m-4)..m+4
    w0 = ((np.maximum(m - R, 0) <= k) & (k <= m + R) & (m < OB)).astype(np.float32)
    # W_int: tile rows = image rows s-4..s+123; out m needs tile k = m..m+8
    wi = ((m <= k) & (k <= m + 2 * R) & (m < OB)).astype(np.float32)
    # W8: tile rows = image rows 896..1023 (128); out m (0..63, global 960+m)
    # needs k = 60+m .. min(68+m, 127)
    w8 = ((m + 64 - R <= k) & (k <= np.minimum(m + 64 + R, 127)) & (m < LASTO)).astype(
        np.float32
    )
    wts = np.stack([w0, wi, w8]).astype(np.float16)

    rowscale = np.empty((128, 3), np.float32)
    rowscale[:, 0] = 1.0 / (ch[0:128] * 9.0)          # block 0 (rows 0..119 used)
    rowscale[:, 1] = 1.0 / 81.0
    rowscale[0:LASTO, 1] = 1.0 / (ch[H - LASTO : H] * 9.0)  # block 8 rows at p 0..63
    rowscale[:, 2] = 1.0 / 81.0
    return wts, rowscale


def _build(reps: int = 1):
    import concourse.bacc as bacc
    import concourse.mybir as mybir
    import concourse.tile as tile

    f32 = mybir.dt.float32
    f16 = mybir.dt.float16

    nc = bacc.Bacc("TRN2", target_bir_lowering=False, debug=False, num_devices=NCORES)
    x_d = nc.declare_dram_parameter("x", [IMGS_PER_CORE, H, W], f32, isOutput=False)
    wts_d = nc.declare_dram_parameter("wts", [3, 128, 128], f16, isOutput=False)
    rs_d = nc.declare_dram_parameter("rowscale", [128, 3], f32, isOutput=False)
    o_d = nc.declare_dram_parameter("out", [IMGS_PER_CORE, H, W], f16, isOutput=True)

    with tile.TileContext(nc) as tc:
        with (
            tc.tile_pool(name="consts", bufs=1) as cpool,
            tc.tile_pool(name="xb", bufs=4) as xb_pool,
            tc.tile_pool(name="e1", bufs=3) as e1_pool,
            tc.tile_pool(name="e2", bufs=4) as e2_pool,
            tc.tile_pool(name="osb", bufs=4) as out_pool,
            tc.tile_pool(name="ps", bufs=4, space="PSUM") as ps_pool,
        ):
            w_sb = cpool.tile([128, 3 * 128], f16)
            for i in range(3):
                nc.sync.dma_start(out=w_sb[:, 128 * i : 128 * (i + 1)], in_=wts_d[i])
            rs_sb = cpool.tile([128, 3], f32)
            nc.sync.dma_start(out=rs_sb[:], in_=rs_d[:])

            def block(g, blk):
                os_, orows, is_, irows, wi, si = blk
                xb = xb_pool.tile([128, WPAD], f16, tag="xb")
                nc.gpsimd.memset(xb[0:irows, 0:4], 0.0)
                nc.gpsimd.memset(xb[0:irows, W + 4 : WPAD], 0.0)
                nc.gpsimd.dma_start(
                    out=xb[0:irows, 4 : W + 4], in_=x_d[g, is_ : is_ + irows, :]
                )
                e1 = e1_pool.tile([128, WPAD - 2], f16, tag="e1")
                nc.vector.tensor_add(
                    out=e1[0:irows, :],
                    in0=xb[0:irows, 0 : WPAD - 2],
                    in1=xb[0:irows, 2:WPAD],
                )
                e2 = e2_pool.tile([128, WPAD - 6], f16, tag="e2")
                nc.vector.tensor_add(
                    out=e2[0:irows, :],
                    in0=e1[0:irows, 0 : WPAD - 6],
                    in1=e1[0:irows, 4 : WPAD - 2],
                )

                out_sb = out_pool.tile([128, W], f16, tag="osb")
                ps = ps_pool.tile([128, 1024], f32, tag="ps")
                wv = w_sb[0:irows, 128 * wi : 128 * wi + orows]
                for h in range(2):
                    j0 = 512 * h
                    nc.tensor.matmul(
                        ps[0:orows, j0 : j0 + 512],
                        wv,
                        e2[0:irows, j0 : j0 + 512],
                        start=True,
                        stop=False,
                    )
                    nc.tensor.matmul(
                        ps[0:orows, j0 : j0 + 512],
                        wv,
                        e2[0:irows, j0 + 1 : j0 + 513],
                        start=False,
                        stop=False,
                    )
                    nc.tensor.matmul(
                        ps[0:orows, j0 : j0 + 512],
                        wv,
                        xb[0:irows, j0 + 8 : j0 + 520],
                        start=False,
                        stop=True,
                    )
                rsv = rs_sb[0:orows, si : si + 1]
                nc.scalar.mul(out_sb[0:orows, :], ps[0:orows, :], rsv)
                nc.scalar.dma_start(
                    out=o_d[g, os_ : os_ + orows, :], in_=out_sb[0:orows, :]
                )

            for _ in range(reps):
                for g in range(IMGS_PER_CORE):
                    for blk in BLOCKS:
                        block(g, blk)

    nc.compile()
    return nc


_LOCK = threading.Lock()
_CACHED = {}


def _get_nc(reps: int = 1):
    with _LOCK:
        key = ("nc", reps)
        if key not in _CACHED:
            _CACHED[key] = _build(reps)
        return _CACHED[key]


def _postprocess(out48_f16: np.ndarray) -> np.ndarray:
    out = out48_f16.astype(np.float32).reshape(B, C, H, W)
    ch = _window_counts()
    out[..., 0:R] *= (9.0 / ch[0:R])[None, None, None, :]
    out[..., W - R : W] *= (9.0 / ch[H - R : H])[None, None, None, :]
    return out


def run(x: np.ndarray, trace: bool = False, reps: int = 1):
    from concourse.bass_utils import run_bass_kernel_spmd

    assert x.shape == (B, C, H, W), x.shape
    x48 = np.ascontiguousarray(x.reshape(IMGS, H, W), dtype=np.float32)
    wts, rowscale = _consts()
    in_maps = [
        {
            "x": np.ascontiguousarray(
                x48[IMGS_PER_CORE * c : IMGS_PER_CORE * (c + 1)]
            ),
            "wts": wts,
            "rowscale": rowscale,
        }
        for c in range(NCORES)
    ]
    nc = _get_nc(reps)
    res = run_bass_kernel_spmd(
        nc, in_maps, core_ids=list(range(NCORES)), trace=trace
    )
    out48 = np.concatenate([r["out"] for r in res.results], axis=0)
    return _postprocess(out48), res


def kernel(x: np.ndarray) -> np.ndarray:
    out, _ = run(x, trace=False)
    return out


# revision 11
# speedup vs baseline: 1.3581x; 1.0763x over previous
"""BoxFilter 9x9 mean, TRN2 x8 — v5: overlapping input tiles, no neighbor MMs.

Each output block of <=120 rows is produced from ONE 128-row (or smaller,
clamped at image edges) input tile that already contains the +/-4-row halo.
Per psum half just 3 matmuls against one band weight: rhs = e2[j], e2[j+1]
(odd shift folded into the slice), xb[j+8]. DVE tree is only 2 ops
(e1={0,2}, e2={0,2,4,6}). One [rows,1024] 2-bank PSUM tile per block, one
ScalarE normalize+cast copy, one output DMA.
"""

import threading

import numpy as np

NCORES = 8
B, C, H, W = 16, 3, 1024, 1024
IMGS = B * C
IMGS_PER_CORE = IMGS // NCORES
R = 4
OB = 120  # output rows per full block
NFULL = H // OB  # 8 full blocks
LASTO = H - NFULL * OB  # 64
WPAD = W + 16

# per-image block table: (out_start, out_rows, in_start, in_rows, w_idx, rs_idx)
BLOCKS = []
BLOCKS.append((0, OB, 0, 128, 0, 0))
for I in range(1, NFULL):
    BLOCKS.append((OB * I, OB, OB * I - R, 128, 1, 2))
BLOCKS.append((H - LASTO, LASTO, H - 128, 128, 2, 1))


def _window_counts():
    r = np.arange(H)
    return (np.minimum(r + R, H - 1) - np.maximum(r - R, 0) + 1).astype(np.float32)


def _consts():
    ch = _window_counts()
    k = np.arange(128)[:, None]
    m = np.arange(128)[None, :]
    # W0: tile rows = image rows 0..127; out m needs rows max(0,m-4)..m+4
    w0 = ((np.maximum(m - R, 0) <= k) & (k <= m + R) & (m < OB)).astype(np.float32)
    # W_int: tile rows = image rows s-4..s+123; out m needs tile k = m..m+8
    wi = ((m <= k) & (k <= m + 2 * R) & (m < OB)).astype(np.float32)
    # W8: tile rows = image rows 896..1023 (128); out m (0..63, global 960+m)
    # needs k = 60+m .. min(68+m, 127)
    w8 = ((m + 64 - R <= k) & (k <= np.minimum(m + 64 + R, 127)) & (m < LASTO)).astype(
        np.float32
    )
    wts = np.stack([w0, wi, w8]).astype(np.float16)

    rowscale = np.empty((128, 3), np.float32)
    rowscale[:, 0] = 1.0 / (ch[0:128] * 9.0)          # block 0 (rows 0..119 used)
    rowscale[:, 1] = 1.0 / 81.0
    rowscale[0:LASTO, 1] = 1.0 / (ch[H - LASTO : H] * 9.0)  # block 8 rows at p 0..63
    rowscale[:, 2] = 1.0 / 81.0
    return wts, rowscale


def _build(reps: int = 1):
    import concourse.bacc as bacc
    import concourse.mybir as mybir
    import concourse.tile as tile

    f32 = mybir.dt.float32
    f16 = mybir.dt.float16

    nc = bacc.Bacc("TRN2", target_bir_lowering=False, debug=False, num_devices=NCORES)
    x_d = nc.declare_dram_parameter("x", [IMGS_PER_CORE, H, W], f32, isOutput=False)
    wts_d = nc.declare_dram_parameter("wts", [3, 128, 128], f16, isOutput=False)
    rs_d = nc.declare_dram_parameter("rowscale", [128, 3], f32, isOutput=False)
    o_d = nc.declare_dram_parameter("out", [IMGS_PER_CORE, H, W], f16, isOutput=True)

    with tile.TileContext(nc) as tc:
        with (
            tc.tile_pool(name="consts", bufs=1) as cpool,
            tc.tile_pool(name="xb", bufs=6) as xb_pool,
            tc.tile_pool(name="e1", bufs=4) as e1_pool,
            tc.tile_pool(name="e2", bufs=6) as e2_pool,
            tc.tile_pool(name="osb", bufs=6) as out_pool,
            tc.tile_pool(name="ps", bufs=4, space="PSUM") as ps_pool,
        ):
            w_sb = cpool.tile([128, 3 * 128], f16)
            for i in range(3):
                nc.sync.dma_start(out=w_sb[:, 128 * i : 128 * (i + 1)], in_=wts_d[i])
            rs_sb = cpool.tile([128, 3], f32)
            nc.sync.dma_start(out=rs_sb[:], in_=rs_d[:])

            def block(g, blk):
                os_, orows, is_, irows, wi, si = blk
                xb = xb_pool.tile([128, WPAD], f16, tag="xb")
                nc.gpsimd.memset(xb[0:irows, 0:4], 0.0)
                nc.gpsimd.memset(xb[0:irows, W + 4 : WPAD], 0.0)
                nc.gpsimd.dma_start(
                    out=xb[0:irows, 4 : W + 4], in_=x_d[g, is_ : is_ + irows, :]
                )
                e1 = e1_pool.tile([128, WPAD - 2], f16, tag="e1")
                nc.vector.tensor_add(
                    out=e1[0:irows, :],
                    in0=xb[0:irows, 0 : WPAD - 2],
                    in1=xb[0:irows, 2:WPAD],
                )
                e2 = e2_pool.tile([128, WPAD - 6], f16, tag="e2")
                nc.vector.tensor_add(
                    out=e2[0:irows, :],
                    in0=e1[0:irows, 0 : WPAD - 6],
                    in1=e1[0:irows, 4 : WPAD - 2],
                )

                out_sb = out_pool.tile([128, W], f16, tag="osb")
                ps = ps_pool.tile([128, 1024], f32, tag="ps")
                wv = w_sb[0:irows, 128 * wi : 128 * wi + orows]
                for h in range(2):
                    j0 = 512 * h
                    nc.tensor.matmul(
                        ps[0:orows, j0 : j0 + 512],
                        wv,
                        e2[0:irows, j0 : j0 + 512],
                        start=True,
                        stop=False,
                    )
                    nc.tensor.matmul(
                        ps[0:orows, j0 : j0 + 512],
                        wv,
                        e2[0:irows, j0 + 1 : j0 + 513],
                        start=False,
                        stop=False,
                    )
                    nc.tensor.matmul(
                        ps[0:orows, j0 : j0 + 512],
                        wv,
                        xb[0:irows, j0 + 8 : j0 + 520],
                        start=False,
                        stop=True,
                    )
                rsv = rs_sb[0:orows, si : si + 1]
                nc.scalar.mul(out_sb[0:orows, :], ps[0:orows, :], rsv)
                nc.scalar.dma_start(
                    out=o_d[g, os_ : os_ + orows, :], in_=out_sb[0:orows, :]
                )

            for _ in range(reps):
                for g in range(IMGS_PER_CORE):
                    for blk in BLOCKS:
                        block(g, blk)

    nc.compile()
    return nc


_LOCK = threading.Lock()
_CACHED = {}


def _get_nc(reps: int = 1):
    with _LOCK:
        key = ("nc", reps)
        if key not in _CACHED:
            _CACHED[key] = _build(reps)
        return _CACHED[key]


def _postprocess(out48_f16: np.ndarray) -> np.ndarray:
    out = out48_f16.astype(np.float32).reshape(B, C, H, W)
    ch = _window_counts()
    out[..., 0:R] *= (9.0 / ch[0:R])[None, None, None, :]
    out[..., W - R : W] *= (9.0 / ch[H - R : H])[None, None, None, :]
    return out


def run(x: np.ndarray, trace: bool = False, reps: int = 1):
    from concourse.bass_utils import run_bass_kernel_spmd

    assert x.shape == (B, C, H, W), x.shape
    x48 = np.ascontiguousarray(x.reshape(IMGS, H, W), dtype=np.float32)
    wts, rowscale = _consts()
    in_maps = [
        {
            "x": np.ascontiguousarray(
                x48[IMGS_PER_CORE * c : IMGS_PER_CORE * (c + 1)]
            ),
            "wts": wts,
            "rowscale": rowscale,
        }
        for c in range(NCORES)
    ]
    nc = _get_nc(reps)
    res = run_bass_kernel_spmd(
        nc, in_maps, core_ids=list(range(NCORES)), trace=trace
    )
    out48 = np.concatenate([r["out"] for r in res.results], axis=0)
    return _postprocess(out48), res


def kernel(x: np.ndarray) -> np.ndarray:
    out, _ = run(x, trace=False)
    return out


# revision 12
# speedup vs baseline: 1.3582x; 1.0000x over previous
"""BoxFilter 9x9 mean, TRN2 x8 — v5: overlapping input tiles, no neighbor MMs.

Each output block of <=120 rows is produced from ONE 128-row (or smaller,
clamped at image edges) input tile that already contains the +/-4-row halo.
Per psum half just 3 matmuls against one band weight: rhs = e2[j], e2[j+1]
(odd shift folded into the slice), xb[j+8]. DVE tree is only 2 ops
(e1={0,2}, e2={0,2,4,6}). One [rows,1024] 2-bank PSUM tile per block, one
ScalarE normalize+cast copy, one output DMA.
"""

import threading

import numpy as np

NCORES = 8
B, C, H, W = 16, 3, 1024, 1024
IMGS = B * C
IMGS_PER_CORE = IMGS // NCORES
R = 4
OB = 120  # output rows per full block
NFULL = H // OB  # 8 full blocks
LASTO = H - NFULL * OB  # 64
WPAD = W + 16

# per-image block table: (out_start, out_rows, in_start, in_rows, w_idx, rs_idx)
BLOCKS = []
BLOCKS.append((0, OB, 0, 124, 0, 0))
for I in range(1, NFULL):
    BLOCKS.append((OB * I, OB, OB * I - R, 128, 1, 2))
BLOCKS.append((H - LASTO, LASTO, H - 96, 96, 2, 1))


def _window_counts():
    r = np.arange(H)
    return (np.minimum(r + R, H - 1) - np.maximum(r - R, 0) + 1).astype(np.float32)


def _consts():
    ch = _window_counts()
    k = np.arange(128)[:, None]
    m = np.arange(128)[None, :]
    # W0: tile rows = image rows 0..127; out m needs rows max(0,m-4)..m+4
    w0 = ((np.maximum(m - R, 0) <= k) & (k <= m + R) & (m < OB)).astype(np.float32)
    # W_int: tile rows = image rows s-4..s+123; out m needs tile k = m..m+8
    wi = ((m <= k) & (k <= m + 2 * R) & (m < OB)).astype(np.float32)
    # W8: tile rows = image rows 928..1023 (96); out m (0..63, global 960+m)
    # needs k = 28+m .. min(36+m, 95)
    w8 = ((m + 32 - R <= k) & (k <= np.minimum(m + 32 + R, 95)) & (m < LASTO)).astype(
        np.float32
    )
    wts = np.stack([w0, wi, w8]).astype(np.float16)

    rowscale = np.empty((128, 3), np.float32)
    rowscale[:, 0] = 1.0 / (ch[0:128] * 9.0)          # block 0 (rows 0..119 used)
    rowscale[:, 1] = 1.0 / 81.0
    rowscale[0:LASTO, 1] = 1.0 / (ch[H - LASTO : H] * 9.0)  # block 8 rows at p 0..63
    rowscale[:, 2] = 1.0 / 81.0
    return wts, rowscale


def _build(reps: int = 1):
    import concourse.bacc as bacc
    import concourse.mybir as mybir
    import concourse.tile as tile

    f32 = mybir.dt.float32
    f16 = mybir.dt.float16

    nc = bacc.Bacc("TRN2", target_bir_lowering=False, debug=False, num_devices=NCORES)
    x_d = nc.declare_dram_parameter("x", [IMGS_PER_CORE, H, W], f32, isOutput=False)
    wts_d = nc.declare_dram_parameter("wts", [3, 128, 128], f16, isOutput=False)
    rs_d = nc.declare_dram_parameter("rowscale", [128, 3], f32, isOutput=False)
    o_d = nc.declare_dram_parameter("out", [IMGS_PER_CORE, H, W], f16, isOutput=True)

    with tile.TileContext(nc) as tc:
        with (
            tc.tile_pool(name="consts", bufs=1) as cpool,
            tc.tile_pool(name="xb", bufs=6) as xb_pool,
            tc.tile_pool(name="e1", bufs=4) as e1_pool,
            tc.tile_pool(name="e2", bufs=6) as e2_pool,
            tc.tile_pool(name="osb", bufs=6) as out_pool,
            tc.tile_pool(name="ps", bufs=4, space="PSUM") as ps_pool,
        ):
            w_sb = cpool.tile([128, 3 * 128], f16)
            for i in range(3):
                nc.sync.dma_start(out=w_sb[:, 128 * i : 128 * (i + 1)], in_=wts_d[i])
            rs_sb = cpool.tile([128, 3], f32)
            nc.sync.dma_start(out=rs_sb[:], in_=rs_d[:])

            def block(g, blk):
                os_, orows, is_, irows, wi, si = blk
                xb = xb_pool.tile([128, WPAD], f16, tag="xb")
                nc.gpsimd.memset(xb[0:irows, 0:4], 0.0)
                nc.gpsimd.memset(xb[0:irows, W + 4 : WPAD], 0.0)
                nc.gpsimd.dma_start(
                    out=xb[0:irows, 4 : W + 4], in_=x_d[g, is_ : is_ + irows, :]
                )
                e1 = e1_pool.tile([128, WPAD - 2], f16, tag="e1")
                nc.vector.tensor_add(
                    out=e1[0:irows, :],
                    in0=xb[0:irows, 0 : WPAD - 2],
                    in1=xb[0:irows, 2:WPAD],
                )
                e2 = e2_pool.tile([128, WPAD - 6], f16, tag="e2")
                nc.vector.tensor_add(
                    out=e2[0:irows, :],
                    in0=e1[0:irows, 0 : WPAD - 6],
                    in1=e1[0:irows, 4 : WPAD - 2],
                )

                out_sb = out_pool.tile([128, W], f16, tag="osb")
                ps = ps_pool.tile([128, 1024], f32, tag="ps")
                wv = w_sb[0:irows, 128 * wi : 128 * wi + orows]
                for h in range(2):
                    j0 = 512 * h
                    nc.tensor.matmul(
                        ps[0:orows, j0 : j0 + 512],
                        wv,
                        e2[0:irows, j0 : j0 + 512],
                        start=True,
                        stop=False,
                    )
                    nc.tensor.matmul(
                        ps[0:orows, j0 : j0 + 512],
                        wv,
                        e2[0:irows, j0 + 1 : j0 + 513],
                        start=False,
                        stop=False,
                    )
                    nc.tensor.matmul(
                        ps[0:orows, j0 : j0 + 512],
                        wv,
                        xb[0:irows, j0 + 8 : j0 + 520],
                        start=False,
                        stop=True,
                    )
                rsv = rs_sb[0:orows, si : si + 1]
                nc.scalar.mul(out_sb[0:orows, :], ps[0:orows, :], rsv)
                nc.scalar.dma_start(
                    out=o_d[g, os_ : os_ + orows, :], in_=out_sb[0:orows, :]
                )

            for _ in range(reps):
                for g in range(IMGS_PER_CORE):
                    for blk in BLOCKS:
                        block(g, blk)

    nc.compile()
    return nc


_LOCK = threading.Lock()
_CACHED = {}


def _get_nc(reps: int = 1):
    with _LOCK:
        key = ("nc", reps)
        if key not in _CACHED:
            _CACHED[key] = _build(reps)
        return _CACHED[key]


def _postprocess(out48_f16: np.ndarray) -> np.ndarray:
    out = out48_f16.astype(np.float32).reshape(B, C, H, W)
    ch = _window_counts()
    out[..., 0:R] *= (9.0 / ch[0:R])[None, None, None, :]
    out[..., W - R : W] *= (9.0 / ch[H - R : H])[None, None, None, :]
    return out


def run(x: np.ndarray, trace: bool = False, reps: int = 1):
    from concourse.bass_utils import run_bass_kernel_spmd

    assert x.shape == (B, C, H, W), x.shape
    x48 = np.ascontiguousarray(x.reshape(IMGS, H, W), dtype=np.float32)
    wts, rowscale = _consts()
    in_maps = [
        {
            "x": np.ascontiguousarray(
                x48[IMGS_PER_CORE * c : IMGS_PER_CORE * (c + 1)]
            ),
            "wts": wts,
            "rowscale": rowscale,
        }
        for c in range(NCORES)
    ]
    nc = _get_nc(reps)
    res = run_bass_kernel_spmd(
        nc, in_maps, core_ids=list(range(NCORES)), trace=trace
    )
    out48 = np.concatenate([r["out"] for r in res.results], axis=0)
    return _postprocess(out48), res


def kernel(x: np.ndarray) -> np.ndarray:
    out, _ = run(x, trace=False)
    return out


# revision 13
# speedup vs baseline: 1.3646x; 1.0048x over previous
"""BoxFilter 9x9 mean, TRN2 x8 — v5: overlapping input tiles, no neighbor MMs.

Each output block of <=120 rows is produced from ONE 128-row (or smaller,
clamped at image edges) input tile that already contains the +/-4-row halo.
Per psum half just 3 matmuls against one band weight: rhs = e2[j], e2[j+1]
(odd shift folded into the slice), xb[j+8]. DVE tree is only 2 ops
(e1={0,2}, e2={0,2,4,6}). One [rows,1024] 2-bank PSUM tile per block, one
ScalarE normalize+cast copy, one output DMA.
"""

import threading

import numpy as np

NCORES = 8
B, C, H, W = 16, 3, 1024, 1024
IMGS = B * C
IMGS_PER_CORE = IMGS // NCORES
R = 4
OB = 120  # output rows per full block
NFULL = H // OB  # 8 full blocks
LASTO = H - NFULL * OB  # 64
WPAD = W + 16

# per-image block table: (out_start, out_rows, in_start, in_rows, w_idx, rs_idx)
BLOCKS = []
BLOCKS.append((0, OB, 0, 124, 0, 0))
for I in range(1, NFULL):
    BLOCKS.append((OB * I, OB, OB * I - R, 128, 1, 2))
BLOCKS.append((H - LASTO, LASTO, H - 96, 96, 2, 1))


def _window_counts():
    r = np.arange(H)
    return (np.minimum(r + R, H - 1) - np.maximum(r - R, 0) + 1).astype(np.float32)


def _consts():
    ch = _window_counts()
    k = np.arange(128)[:, None]
    m = np.arange(128)[None, :]
    # W0: tile rows = image rows 0..127; out m needs rows max(0,m-4)..m+4
    w0 = ((np.maximum(m - R, 0) <= k) & (k <= m + R) & (m < OB)).astype(np.float32)
    # W_int: tile rows = image rows s-4..s+123; out m needs tile k = m..m+8
    wi = ((m <= k) & (k <= m + 2 * R) & (m < OB)).astype(np.float32)
    # W8: tile rows = image rows 928..1023 (96); out m (0..63, global 960+m)
    # needs k = 28+m .. min(36+m, 95)
    w8 = ((m + 32 - R <= k) & (k <= np.minimum(m + 32 + R, 95)) & (m < LASTO)).astype(
        np.float32
    )
    wts = np.stack([w0, wi, w8]).astype(np.float16)

    rowscale = np.empty((128, 3), np.float32)
    rowscale[:, 0] = 1.0 / (ch[0:128] * 9.0)          # block 0 (rows 0..119 used)
    rowscale[:, 1] = 1.0 / 81.0
    rowscale[0:LASTO, 1] = 1.0 / (ch[H - LASTO : H] * 9.0)  # block 8 rows at p 0..63
    rowscale[:, 2] = 1.0 / 81.0
    return wts, rowscale


def _build(reps: int = 1):
    import concourse.bacc as bacc
    import concourse.mybir as mybir
    import concourse.tile as tile

    f32 = mybir.dt.float32
    f16 = mybir.dt.float16

    nc = bacc.Bacc("TRN2", target_bir_lowering=False, debug=False, num_devices=NCORES)
    x_d = nc.declare_dram_parameter("x", [IMGS_PER_CORE, H, W], f32, isOutput=False)
    wts_d = nc.declare_dram_parameter("wts", [3, 128, 128], f16, isOutput=False)
    rs_d = nc.declare_dram_parameter("rowscale", [128, 3], f32, isOutput=False)
    o_d = nc.declare_dram_parameter("out", [IMGS_PER_CORE, H, W], f16, isOutput=True)

    with tile.TileContext(nc) as tc:
        with (
            tc.tile_pool(name="consts", bufs=1) as cpool,
            tc.tile_pool(name="xb", bufs=6) as xb_pool,
            tc.tile_pool(name="e1", bufs=4) as e1_pool,
            tc.tile_pool(name="e2", bufs=6) as e2_pool,
            tc.tile_pool(name="osb", bufs=6) as out_pool,
            tc.tile_pool(name="ps", bufs=8, space="PSUM") as ps_pool,
        ):
            w_sb = cpool.tile([128, 3 * 128], f16)
            for i in range(3):
                nc.sync.dma_start(out=w_sb[:, 128 * i : 128 * (i + 1)], in_=wts_d[i])
            rs_sb = cpool.tile([128, 3], f32)
            nc.sync.dma_start(out=rs_sb[:], in_=rs_d[:])

            def block(g, blk):
                os_, orows, is_, irows, wi, si = blk
                xb = xb_pool.tile([128, WPAD], f16, tag="xb")
                nc.gpsimd.memset(xb[0:irows, 0:4], 0.0)
                nc.gpsimd.memset(xb[0:irows, W + 4 : WPAD], 0.0)
                nc.gpsimd.dma_start(
                    out=xb[0:irows, 4 : W + 4], in_=x_d[g, is_ : is_ + irows, :]
                )
                e1 = e1_pool.tile([128, WPAD - 2], f16, tag="e1")
                nc.vector.tensor_add(
                    out=e1[0:irows, :],
                    in0=xb[0:irows, 0 : WPAD - 2],
                    in1=xb[0:irows, 2:WPAD],
                )
                e2 = e2_pool.tile([128, WPAD - 6], f16, tag="e2")
                nc.vector.tensor_add(
                    out=e2[0:irows, :],
                    in0=e1[0:irows, 0 : WPAD - 6],
                    in1=e1[0:irows, 4 : WPAD - 2],
                )

                out_sb = out_pool.tile([128, W], f16, tag="osb")
                wv = w_sb[0:irows, 128 * wi : 128 * wi + orows]
                rsv = rs_sb[0:orows, si : si + 1]
                for h in range(2):
                    j0 = 512 * h
                    ps = ps_pool.tile([128, 512], f32, tag="ps", name=f"ps{h}")
                    nc.tensor.matmul(
                        ps[0:orows, :],
                        wv,
                        e2[0:irows, j0 : j0 + 512],
                        start=True,
                        stop=False,
                    )
                    nc.tensor.matmul(
                        ps[0:orows, :],
                        wv,
                        e2[0:irows, j0 + 1 : j0 + 513],
                        start=False,
                        stop=False,
                    )
                    nc.tensor.matmul(
                        ps[0:orows, :],
                        wv,
                        xb[0:irows, j0 + 8 : j0 + 520],
                        start=False,
                        stop=True,
                    )
                    nc.scalar.mul(
                        out_sb[0:orows, j0 : j0 + 512], ps[0:orows, :], rsv
                    )
                nc.scalar.dma_start(
                    out=o_d[g, os_ : os_ + orows, :], in_=out_sb[0:orows, :]
                )

            for _ in range(reps):
                for g in range(IMGS_PER_CORE):
                    for blk in BLOCKS:
                        block(g, blk)

    nc.compile()
    return nc


_LOCK = threading.Lock()
_CACHED = {}


def _get_nc(reps: int = 1):
    with _LOCK:
        key = ("nc", reps)
        if key not in _CACHED:
            _CACHED[key] = _build(reps)
        return _CACHED[key]


def _postprocess(out48_f16: np.ndarray) -> np.ndarray:
    out = out48_f16.astype(np.float32).reshape(B, C, H, W)
    ch = _window_counts()
    out[..., 0:R] *= (9.0 / ch[0:R])[None, None, None, :]
    out[..., W - R : W] *= (9.0 / ch[H - R : H])[None, None, None, :]
    return out


def run(x: np.ndarray, trace: bool = False, reps: int = 1):
    from concourse.bass_utils import run_bass_kernel_spmd

    assert x.shape == (B, C, H, W), x.shape
    x48 = np.ascontiguousarray(x.reshape(IMGS, H, W), dtype=np.float32)
    wts, rowscale = _consts()
    in_maps = [
        {
            "x": np.ascontiguousarray(
                x48[IMGS_PER_CORE * c : IMGS_PER_CORE * (c + 1)]
            ),
            "wts": wts,
            "rowscale": rowscale,
        }
        for c in range(NCORES)
    ]
    nc = _get_nc(reps)
    res = run_bass_kernel_spmd(
        nc, in_maps, core_ids=list(range(NCORES)), trace=trace
    )
    out48 = np.concatenate([r["out"] for r in res.results], axis=0)
    return _postprocess(out48), res


def kernel(x: np.ndarray) -> np.ndarray:
    out, _ = run(x, trace=False)
    return out
